# revision 36
# baseline (speedup 1.0000x reference)
"""ADM attention block (B=4, C=512, H=W=64) on 8 TRN2 NeuronCores.

Sharding: core = (b, half) = (core//2, core%2). Data-parallel over batch (4)
x query-halves (2), zero collectives. The query half is selected on the host
by permuting the N axis of x so "my" queries are always columns 0:2048.

v2: weight-norm folded into host preprocessing; all heavy matmuls run in
fp8e4 with perf_mode=DoubleRow (K=256 per instruction, 2x PE throughput);
h accumulates transposed ([c, i] in PSUM) so the epilogue needs no PE
transposes; the softmax denominator comes from one DoubleRow ones-matmul
per j-pair into a [128,512] PSUM tile (M=128 -> replicated across
partitions), normalized via a broadcast reciprocal multiply. Vector work is
load-balanced across DVE (nc.vector) and Pool (nc.gpsimd).

Numerics: w_qkv is host-scaled by S=16 before the fp8 cast (RMS divide is
scale-invariant; the post-sqrt eps is compensated exactly by adding S*eps).
exp carries a -4ln2 bias so fp8 ex stays below the TRN e4m3 max of 240;
the 2^-4 factor cancels in h/den. The residual path stays f32 end-to-end.
"""

import os
from contextlib import ExitStack

import numpy as np
import ml_dtypes

import concourse.bass as bass
import concourse.mybir as mybir
import concourse.tile as tile
from concourse.bass_utils import run_bass_kernel_spmd

B, C, N = 4, 512, 4096
NH = N // 2
P = 128
O3 = 3 * C             # 1536
NCH = N // P           # 32 n-chunks
QCH = NH // P          # 16 query chunks per core
T = NCH // 2           # 16 j-pairs (DoubleRow contracts 256 keys at once)
ISL = 512              # query i-slice
NISL = NH // ISL       # 4 i-slices
S = 16.0               # host weight scale for fp8
SG = 16.0              # host scale for the Gram matrix G = W^T W
F32 = mybir.dt.float32
BF16 = mybir.dt.bfloat16
F8 = mybir.dt.float8e4
DR = mybir.MatmulPerfMode.DoubleRow
EXP_BIAS = -2.772588722239781  # -4*ln(2): keeps fp8 ex <= ~15 << 240

LAST_RESULT = None

_TPB_ENGINES = (
    mybir.EngineType.PE,
    mybir.EngineType.Activation,
    mybir.EngineType.DVE,
    mybir.EngineType.Pool,
    mybir.EngineType.SP,
)


def _split_waits(nc):
    """walrus on this image rejects >1 sem-wait on a TPB instruction. Hoist
    excess waits onto engine-local NoOps, each carrying one wait."""
    ctr = 0
    for fn in nc.m.functions:
        for blk in fn.blocks:
            new_insts = []
            for inst in blk.instructions:
                si = getattr(inst, "sync_info", None)
                eng = getattr(inst, "engine", None)
                if (
                    si is not None
                    and si.on_wait
                    and len(si.on_wait) > 1
                    and eng in _TPB_ENGINES
                ):
                    for sw in si.on_wait[:-1]:
                        ctr += 1
                        nop = mybir.InstNoOp(
                            name=f"wsplit-{ctr}", engine=eng, ins=[], outs=[],
                            sync_info=mybir.SyncInfo(on_wait=[sw], on_update=[]),
                        )
                        new_insts.append(nop)
                    inst.sync_info = mybir.SyncInfo(
                        on_wait=[si.on_wait[-1]], on_update=si.on_update,
                    )
                new_insts.append(inst)
            blk.instructions[:] = new_insts


def build_graph():
    nc = bass.Bass()

    x_pack_d = nc.declare_dram_parameter("x_pack", [P, 4, N], F8, isOutput=False)
    xn_pack_d = nc.declare_dram_parameter("xn_pack", [P, NCH, C], BF16, isOutput=False)
    w_pack_d = nc.declare_dram_parameter("w_pack", [P, 2, 2, O3], F8, isOutput=False)
    g_pack_d = nc.declare_dram_parameter("g_pack", [P, 2, 2, C], F8, isOutput=False)
    wo_pack_d = nc.declare_dram_parameter("wo_pack", [P, 4, C], BF16, isOutput=False)
    ident_d = nc.declare_dram_parameter("ident_bf", [P, P], BF16, isOutput=False)
    xt_nc = nc.declare_dram_parameter("xt_nc", [NH, C], F32, isOutput=False)
    out_d = nc.declare_dram_parameter("out", [NH, C], F32, isOutput=True)

    with tile.TileContext(nc) as tc, ExitStack() as ctx:
        singles = ctx.enter_context(tc.tile_pool(name="singles", bufs=1))

        w_sb = singles.tile([P, 2, 2, O3], F8)
        nc.sync.dma_start(out=w_sb, in_=w_pack_d[:, :, :, :])
        g_sb = singles.tile([P, 2, 2, C], F8)
        nc.sync.dma_start(out=g_sb, in_=g_pack_d[:, :, :, :])
        wo_sb = singles.tile([P, 4, C], BF16)
        nc.sync.dma_start(out=wo_sb, in_=wo_pack_d[:, :, :])
        ident = singles.tile([P, P], BF16)
        nc.sync.dma_start(out=ident, in_=ident_d[:, :])
        ones2 = singles.tile([P, 2, P], F8)
        nc.vector.memset(ones2, 1.0)
        ebias = singles.tile([P, 1], F32)
        nc.vector.memset(ebias, EXP_BIAS)

        # persistent attention operands
        big = ctx.enter_context(tc.tile_pool(name="big", bufs=1))
        q_sb = big.tile([P, 4, NH], F8)    # q_hat^T: [c-chunk][i]
        k_sb = big.tile([P, 4, N], F8)     # k_hat^T: [c-chunk][j]
        v_sb = big.tile([P, T, 2, C], F8)  # v_hat:   [j-pair][plane][c]

        # ---- phase 1: QKV (fp8 DoubleRow) + RMS + operand builds ----
        with tc.tile_pool(name="xp", bufs=3) as xp, \
             tc.tile_pool(name="qkvps", bufs=2, space="PSUM") as qkvps, \
             tc.tile_pool(name="yps", bufs=2, space="PSUM") as yps, \
             tc.tile_pool(name="sqp", bufs=2) as sqp, \
             tc.tile_pool(name="rp", bufs=4) as rp, \
             tc.tile_pool(name="qnp", bufs=2) as qnp:
            for nch in range(NCH):
                x_sb = xp.tile([P, 4, P], F8, tag="x_sb")
                nc.sync.dma_start(out=x_sb, in_=x_pack_d[:, :, nch * P:(nch + 1) * P])
                xn_sb = xp.tile([P, C], BF16, tag="xn_sb")
                nc.sync.dma_start(out=xn_sb, in_=xn_pack_d[:, nch, :])
                # y = x^T G with G = W^T W host-precomputed, so
                # sum_o qkv[n,o]^2 = sum_c x[c,n] * y[n,c] -- no ACT square
                ps = qkvps.tile([P, 3, 512], F32, tag="ps")
                y = yps.tile([P, 512], F32, tag="y")
                for c2 in range(2):
                    nc.tensor.matmul(
                        y,
                        lhsT=x_sb[:, 2 * c2:2 * c2 + 2, :],
                        rhs=g_sb[:, c2, :, :],
                        start=(c2 == 0), stop=(c2 == 1),
                        perf_mode=DR,
                    )
                for os_ in range(3):
                    for c2 in range(2):
                        nc.tensor.matmul(
                            ps[:, os_, :],
                            lhsT=x_sb[:, 2 * c2:2 * c2 + 2, :],
                            rhs=w_sb[:, c2, :, os_ * 512:(os_ + 1) * 512],
                            start=(c2 == 0), stop=(c2 == 1),
                            perf_mode=DR,
                        )
                sq = sqp.tile([P, 512], BF16, tag="sq")
                ssum = rp.tile([P, 1], F32, tag="ssum")
                nc.vector.scalar_tensor_tensor(
                    out=sq, in0=y, scalar=1.0, in1=xn_sb,
                    op0=mybir.AluOpType.mult, op1=mybir.AluOpType.mult,
                    accum_out=ssum,
                )
                # r = 1/(S*(rms + eps)); ssum = S*Sg * sum(qkv^2)
                r = rp.tile([P, 1], F32, tag="r")
                nc.scalar.activation(out=r, in_=ssum,
                                     func=mybir.ActivationFunctionType.Sqrt,
                                     scale=S * S / (O3 * SG))
                nc.vector.tensor_scalar_add(r, r, S * 1e-4)
                nc.vector.reciprocal(r, r)
                # v_hat straight into the attention rhs layout (ACT is light
                # here now that the RMS square pass lives on PE/DVE)
                nc.scalar.activation(out=v_sb[:, nch // 2, nch % 2, :],
                                     in_=ps[:, 2, :],
                                     func=mybir.ActivationFunctionType.Copy,
                                     scale=r)
                # q,k normalized to bf16, then PE-transposed to [c, n];
                # the q half is only needed for the first QCH chunks
                qn = qnp.tile([P, 2, 512], BF16, tag="qn")
                if nch < QCH:
                    nc.vector.tensor_scalar_mul(qn, ps[:, 0:2, :], r)
                else:
                    nc.vector.tensor_scalar_mul(qn[:, 1, :], ps[:, 1, :], r)
                # stage the PE transposes in the (already-consumed) y tile,
                # reinterpreted as [P, 1024] bf16: k in cols 0:512, q in 512:1024
                y_bf = y.bitcast(BF16)
                for cc in range(4):
                    nc.tensor.transpose(out=y_bf[:, cc * P:(cc + 1) * P],
                                        in_=qn[:, 1, cc * P:(cc + 1) * P],
                                        identity=ident)
                nc.vector.tensor_copy(out=k_sb[:, :, nch * P:(nch + 1) * P],
                                      in_=y_bf[:, 0:512])
                if nch < QCH:
                    for cc in range(4):
                        nc.tensor.transpose(out=y_bf[:, 512 + cc * P:512 + (cc + 1) * P],
                                            in_=qn[:, 0, cc * P:(cc + 1) * P],
                                            identity=ident)
                    nc.scalar.copy(out=q_sb[:, :, nch * P:(nch + 1) * P],
                                   in_=y_bf[:, 512:1024])

        # ---- phase 2: attention (fp8 DoubleRow), hT accumulation ----
        scp = ctx.enter_context(tc.tile_pool(name="scp", bufs=1, space="PSUM"))
        hps = ctx.enter_context(tc.tile_pool(name="hps", bufs=1, space="PSUM"))
        dps = ctx.enter_context(tc.tile_pool(name="dps", bufs=1, space="PSUM"))
        pop = ctx.enter_context(tc.tile_pool(name="pop", bufs=1, space="PSUM"))
        expp = ctx.enter_context(tc.tile_pool(name="expp", bufs=2))
        rdp = ctx.enter_context(tc.tile_pool(name="rdp", bufs=2))
        htp = ctx.enter_context(tc.tile_pool(name="htp", bufs=2))
        xtp = ctx.enter_context(tc.tile_pool(name="xtp", bufs=3))
        obp = ctx.enter_context(tc.tile_pool(name="obp", bufs=3))

        tiles = {}  # isl -> (h_ps, den_ps), allocated lazily at first acc

        def scores_exp(isl, t):
            sc = scp.tile([P, 2, 512], F32, tag="sc")
            for pl in range(2):
                j = 2 * t + pl
                for c2 in range(2):
                    nc.tensor.matmul(
                        sc[:, pl, :],
                        lhsT=k_sb[:, 2 * c2:2 * c2 + 2, j * P:(j + 1) * P],
                        rhs=q_sb[:, 2 * c2:2 * c2 + 2, isl * ISL:(isl + 1) * ISL],
                        start=(c2 == 0), stop=(c2 == 1),
                        perf_mode=DR,
                    )
            ex = expp.tile([P, 2, 512], F8, tag="ex")
            nc.scalar.activation(out=ex, in_=sc,
                                 func=mybir.ActivationFunctionType.Exp,
                                 scale=float(C) ** -0.5, bias=ebias)
            return ex

        def acc_h_den(isl, t, ex):
            if isl not in tiles:
                h_t = hps.tile([P, 4, 512], F32, tag="h")
                den_t = dps.tile([P, 512], F32, tag="den")
                tiles[isl] = (h_t, den_t)
            h_ps, den_ps = tiles[isl]
            nc.tensor.matmul(
                den_ps, lhsT=ones2, rhs=ex,
                start=(t == 0), stop=(t == T - 1), perf_mode=DR,
            )
            for cc in range(4):
                nc.tensor.matmul(
                    h_ps[:, cc, :],
                    lhsT=v_sb[:, t, :, cc * P:(cc + 1) * P],
                    rhs=ex,
                    start=(t == 0), stop=(t == T - 1),
                    perf_mode=DR,
                )

        def epilogue(isl):
            h_ps, den_ps = tiles.pop(isl)
            rden = rdp.tile([P, 512], F32, tag="rden")
            nc.vector.reciprocal(rden, den_ps)
            hTn = htp.tile([P, 4, 512], BF16, tag="hTn")
            for cc in range(4):
                nc.vector.tensor_mul(hTn[:, cc, :], h_ps[:, cc, :], rden)
            for a in range(4):
                po = pop.tile([P, 512], F32, tag="po")
                for cc in range(4):
                    nc.tensor.matmul(
                        po,
                        lhsT=hTn[:, cc, a * P:(a + 1) * P],
                        rhs=wo_sb[:, cc, :],
                        start=(cc == 0), stop=(cc == 3),
                    )
                ich = isl * 4 + a
                xt_sb = xtp.tile([P, C], F32, tag="xt_sb")
                nc.sync.dma_start(out=xt_sb, in_=xt_nc[ich * P:(ich + 1) * P, :])
                ob = obp.tile([P, C], F32, tag="ob")
                nc.vector.tensor_add(ob, po, xt_sb)
                nc.sync.dma_start(out=out_d[ich * P:(ich + 1) * P, :], in_=ob)

        # flat software pipeline over all (isl, t) pairs: emit scores(t+1)
        # before h/den(t) so the PE computes scores while ACT runs exp; the
        # per-isl epilogue is emitted right after its last h/den lands.
        pairs = [(isl, t) for isl in range(NISL) for t in range(T)]
        prev = None
        for cur in pairs:
            ex_cur = scores_exp(*cur)
            if prev is not None:
                acc_h_den(prev[0], prev[1], ex_prev)
                if prev[1] == T - 1:
                    epilogue(prev[0])
            prev, ex_prev = cur, ex_cur
        acc_h_den(prev[0], prev[1], ex_prev)
        epilogue(prev[0])

    _split_waits(nc)
    return nc


_GRAPH = None


def _f8(a):
    return np.asarray(a, dtype=np.float32).astype(ml_dtypes.float8_e4m3)


def kernel(**inputs):
    global _GRAPH, LAST_RESULT
    x = np.ascontiguousarray(np.asarray(inputs["x"], dtype=np.float32))
    v_qkv = np.asarray(inputs["v_qkv"], dtype=np.float32)
    g_qkv = np.asarray(inputs["g_qkv"], dtype=np.float32)
    v_out = np.asarray(inputs["v_out"], dtype=np.float32)
    g_out = np.asarray(inputs["g_out"], dtype=np.float32)

    # weight norm on host
    w_qkv = (g_qkv[:, None] * v_qkv
             / np.linalg.norm(v_qkv.astype(np.float64), axis=1, keepdims=True)
             ).astype(np.float32)  # [3C, C]
    w_out = (g_out[:, None] * v_out
             / np.linalg.norm(v_out.astype(np.float64), axis=1, keepdims=True)
             ).astype(np.float32)  # [C, C]

    # [128 p, 2 c2, 2 pl, O3]: w_pack[p,c2,pl,o] = S * w_qkv[o, c2*256+pl*128+p]
    wq = (S * w_qkv.T).reshape(2, 2, P, O3)
    w_pack = _f8(np.ascontiguousarray(wq.transpose(2, 0, 1, 3)))
    # Gram matrix for the RMS sum-of-squares: same c_in packing as w_pack
    G = (SG * (w_qkv.T @ w_qkv)).reshape(2, 2, P, C)
    g_pack = _f8(np.ascontiguousarray(G.transpose(2, 0, 1, 3)))
    # [128 p, 4 cc, C]: wo_pack[p,cc,o] = 2^-0.5 * w_out[o, cc*128+p]
    wo = (np.float32(2.0 ** -0.5) * w_out.T).reshape(4, P, C)
    wo_pack = np.ascontiguousarray(wo.transpose(1, 0, 2)).astype(ml_dtypes.bfloat16)
    ident_bf = np.eye(P, dtype=ml_dtypes.bfloat16)
    rsqrt2 = np.float32(2.0 ** -0.5)

    xt = x.reshape(B, C, N)
    in_maps = []
    for core in range(8):
        b, h = core // 2, core % 2
        if h == 0:
            x_perm = xt[b]
        else:
            x_perm = np.concatenate([xt[b][:, NH:], xt[b][:, :NH]], axis=1)
        x_perm = np.ascontiguousarray(x_perm)
        x_pack = np.ascontiguousarray(
            x_perm.reshape(4, P, N).transpose(1, 0, 2))  # [128, 4cc, N]
        xn_pack = np.ascontiguousarray(
            x_perm.T.reshape(NCH, P, C).transpose(1, 0, 2)
        ).astype(ml_dtypes.bfloat16)  # [128 p, nch, C]: x_perm[c, nch*128+p]
        in_maps.append({
            "x_pack": _f8(x_pack),
            "xn_pack": xn_pack,
            "w_pack": w_pack,
            "g_pack": g_pack,
            "wo_pack": wo_pack,
            "ident_bf": ident_bf,
            "xt_nc": np.ascontiguousarray(x_perm[:, :NH].T * rsqrt2),
        })

    if _GRAPH is None:
        _GRAPH = build_graph()

    res = run_bass_kernel_spmd(_GRAPH, in_maps, core_ids=list(range(8)))
    LAST_RESULT = res

    out = np.empty((B, C, N), np.float32)
    for core in range(8):
        b, h = core // 2, core % 2
        out[b][:, h * NH:(h + 1) * NH] = res.results[core]["out"].T
    return out.reshape(B, C, 64, 64)


# revision 37
# speedup vs baseline: 1.0874x; 1.0874x over previous
"""ADM attention block (B=4, C=512, H=W=64) on 8 TRN2 NeuronCores.

Sharding: core = (b, half) = (core//2, core%2). Data-parallel over batch (4)
x query-halves (2), zero collectives. The query half is selected on the host
by permuting the N axis of x so "my" queries are always columns 0:2048.

v2: weight-norm folded into host preprocessing; all heavy matmuls run in
fp8e4 with perf_mode=DoubleRow (K=256 per instruction, 2x PE throughput);
h accumulates transposed ([c, i] in PSUM) so the epilogue needs no PE
transposes; the softmax denominator comes from one DoubleRow ones-matmul
per j-pair into a [128,512] PSUM tile (M=128 -> replicated across
partitions), normalized via a broadcast reciprocal multiply. Vector work is
load-balanced across DVE (nc.vector) and Pool (nc.gpsimd).

Numerics: w_qkv is host-scaled by S=16 before the fp8 cast (RMS divide is
scale-invariant; the post-sqrt eps is compensated exactly by adding S*eps).
exp carries a -4ln2 bias so fp8 ex stays below the TRN e4m3 max of 240;
the 2^-4 factor cancels in h/den. The residual path stays f32 end-to-end.
"""

import os
from contextlib import ExitStack

import numpy as np
import ml_dtypes

import concourse.bass as bass
import concourse.mybir as mybir
import concourse.tile as tile
from concourse.bass_utils import run_bass_kernel_spmd

B, C, N = 4, 512, 4096
NH = N // 2
P = 128
O3 = 3 * C             # 1536
NCH = N // P           # 32 n-chunks
QCH = NH // P          # 16 query chunks per core
T = NCH // 2           # 16 j-pairs (DoubleRow contracts 256 keys at once)
ISL = 512              # query i-slice
NISL = NH // ISL       # 4 i-slices
S = 16.0               # host weight scale for fp8
SG = 16.0              # host scale for the Gram matrix G = W^T W
F32 = mybir.dt.float32
BF16 = mybir.dt.bfloat16
F8 = mybir.dt.float8e4
DR = mybir.MatmulPerfMode.DoubleRow
EXP_BIAS = -2.772588722239781  # -4*ln(2): keeps fp8 ex <= ~15 << 240

LAST_RESULT = None

_TPB_ENGINES = (
    mybir.EngineType.PE,
    mybir.EngineType.Activation,
    mybir.EngineType.DVE,
    mybir.EngineType.Pool,
    mybir.EngineType.SP,
)


def _split_waits(nc):
    """walrus on this image rejects >1 sem-wait on a TPB instruction. Hoist
    excess waits onto engine-local NoOps, each carrying one wait."""
    ctr = 0
    for fn in nc.m.functions:
        for blk in fn.blocks:
            new_insts = []
            for inst in blk.instructions:
                si = getattr(inst, "sync_info", None)
                eng = getattr(inst, "engine", None)
                if (
                    si is not None
                    and si.on_wait
                    and len(si.on_wait) > 1
                    and eng in _TPB_ENGINES
                ):
                    for sw in si.on_wait[:-1]:
                        ctr += 1
                        nop = mybir.InstNoOp(
                            name=f"wsplit-{ctr}", engine=eng, ins=[], outs=[],
                            sync_info=mybir.SyncInfo(on_wait=[sw], on_update=[]),
                        )
                        new_insts.append(nop)
                    inst.sync_info = mybir.SyncInfo(
                        on_wait=[si.on_wait[-1]], on_update=si.on_update,
                    )
                new_insts.append(inst)
            blk.instructions[:] = new_insts


def build_graph():
    nc = bass.Bass()

    x_pack_d = nc.declare_dram_parameter("x_pack", [P, 4, N], F8, isOutput=False)
    xn_pack_d = nc.declare_dram_parameter("xn_pack", [P, NCH, C], BF16, isOutput=False)
    w_pack_d = nc.declare_dram_parameter("w_pack", [P, 2, 2, O3], F8, isOutput=False)
    g_pack_d = nc.declare_dram_parameter("g_pack", [P, 2, 2, C], F8, isOutput=False)
    wo_pack_d = nc.declare_dram_parameter("wo_pack", [P, 4, C], BF16, isOutput=False)
    ident_d = nc.declare_dram_parameter("ident_bf", [P, P], BF16, isOutput=False)
    xt_nc = nc.declare_dram_parameter("xt_nc", [NH, C], F32, isOutput=False)
    out_d = nc.declare_dram_parameter("out", [NH, C], F32, isOutput=True)

    with tile.TileContext(nc) as tc, ExitStack() as ctx:
        singles = ctx.enter_context(tc.tile_pool(name="singles", bufs=1))

        w_sb = singles.tile([P, 2, 2, O3], F8)
        nc.sync.dma_start(out=w_sb, in_=w_pack_d[:, :, :, :])
        g_sb = singles.tile([P, 2, 2, C], F8)
        nc.sync.dma_start(out=g_sb, in_=g_pack_d[:, :, :, :])
        wo_sb = singles.tile([P, 4, C], BF16)
        nc.sync.dma_start(out=wo_sb, in_=wo_pack_d[:, :, :])
        ident = singles.tile([P, P], BF16)
        nc.sync.dma_start(out=ident, in_=ident_d[:, :])
        ones2 = singles.tile([P, 2, P], F8)
        nc.vector.memset(ones2, 1.0)
        ebias = singles.tile([P, 1], F32)
        nc.vector.memset(ebias, EXP_BIAS)

        # persistent attention operands
        big = ctx.enter_context(tc.tile_pool(name="big", bufs=1))
        q_sb = big.tile([P, 4, NH], F8)    # q_hat^T: [c-chunk][i]
        k_sb = big.tile([P, 4, N], F8)     # k_hat^T: [c-chunk][j]
        v_sb = big.tile([P, T, 2, C], F8)  # v_hat:   [j-pair][plane][c]

        # ---- phase 1: QKV (fp8 DoubleRow) + RMS + operand builds ----
        with tc.tile_pool(name="xp", bufs=3) as xp, \
             tc.tile_pool(name="qkvps", bufs=2, space="PSUM") as qkvps, \
             tc.tile_pool(name="yps", bufs=2, space="PSUM") as yps, \
             tc.tile_pool(name="sqp", bufs=2) as sqp, \
             tc.tile_pool(name="rp", bufs=4) as rp, \
             tc.tile_pool(name="qnp", bufs=2) as qnp:
            for nch in range(NCH):
                x_sb = xp.tile([P, 4, P], F8, tag="x_sb")
                nc.sync.dma_start(out=x_sb, in_=x_pack_d[:, :, nch * P:(nch + 1) * P])
                xn_sb = xp.tile([P, C], BF16, tag="xn_sb")
                nc.sync.dma_start(out=xn_sb, in_=xn_pack_d[:, nch, :])
                # y = x^T G with G = W^T W host-precomputed, so
                # sum_o qkv[n,o]^2 = sum_c x[c,n] * y[n,c] -- no ACT square
                ps = qkvps.tile([P, 3, 512], F32, tag="ps")
                y = yps.tile([P, 512], F32, tag="y")
                for c2 in range(2):
                    nc.tensor.matmul(
                        y,
                        lhsT=x_sb[:, 2 * c2:2 * c2 + 2, :],
                        rhs=g_sb[:, c2, :, :],
                        start=(c2 == 0), stop=(c2 == 1),
                        perf_mode=DR,
                    )
                for os_ in range(3):
                    for c2 in range(2):
                        nc.tensor.matmul(
                            ps[:, os_, :],
                            lhsT=x_sb[:, 2 * c2:2 * c2 + 2, :],
                            rhs=w_sb[:, c2, :, os_ * 512:(os_ + 1) * 512],
                            start=(c2 == 0), stop=(c2 == 1),
                            perf_mode=DR,
                        )
                sq = sqp.tile([P, 512], BF16, tag="sq")
                ssum = rp.tile([P, 1], F32, tag="ssum")
                nc.vector.scalar_tensor_tensor(
                    out=sq, in0=y, scalar=1.0, in1=xn_sb,
                    op0=mybir.AluOpType.mult, op1=mybir.AluOpType.mult,
                    accum_out=ssum,
                )
                # r = 1/(S*(rms + eps)); ssum = S*Sg * sum(qkv^2)
                r = rp.tile([P, 1], F32, tag="r")
                nc.scalar.activation(out=r, in_=ssum,
                                     func=mybir.ActivationFunctionType.Sqrt,
                                     scale=S * S / (O3 * SG))
                nc.vector.tensor_scalar_add(r, r, S * 1e-4)
                nc.vector.reciprocal(r, r)
                # v_hat straight into the attention rhs layout
                nc.vector.tensor_scalar_mul(
                    v_sb[:, nch // 2, nch % 2, :], ps[:, 2, :], r)
                # q,k normalized to bf16, then PE-transposed to [c, n];
                # the q half is only needed for the first QCH chunks
                qn = qnp.tile([P, 2, 512], BF16, tag="qn")
                if nch < QCH:
                    nc.vector.tensor_scalar_mul(qn, ps[:, 0:2, :], r)
                else:
                    nc.vector.tensor_scalar_mul(qn[:, 1, :], ps[:, 1, :], r)
                # stage the PE transposes in the (already-consumed) y tile,
                # reinterpreted as [P, 1024] bf16: k in cols 0:512, q in 512:1024
                y_bf = y.bitcast(BF16)
                for cc in range(4):
                    nc.tensor.transpose(out=y_bf[:, cc * P:(cc + 1) * P],
                                        in_=qn[:, 1, cc * P:(cc + 1) * P],
                                        identity=ident)
                nc.vector.tensor_copy(out=k_sb[:, :, nch * P:(nch + 1) * P],
                                      in_=y_bf[:, 0:512])
                if nch < QCH:
                    for cc in range(4):
                        nc.tensor.transpose(out=y_bf[:, 512 + cc * P:512 + (cc + 1) * P],
                                            in_=qn[:, 0, cc * P:(cc + 1) * P],
                                            identity=ident)
                    nc.scalar.copy(out=q_sb[:, :, nch * P:(nch + 1) * P],
                                   in_=y_bf[:, 512:1024])

        # ---- phase 2: attention (fp8 DoubleRow), hT accumulation ----
        scp = ctx.enter_context(tc.tile_pool(name="scp", bufs=1, space="PSUM"))
        hps = ctx.enter_context(tc.tile_pool(name="hps", bufs=1, space="PSUM"))
        dps = ctx.enter_context(tc.tile_pool(name="dps", bufs=1, space="PSUM"))
        pop = ctx.enter_context(tc.tile_pool(name="pop", bufs=1, space="PSUM"))
        expp = ctx.enter_context(tc.tile_pool(name="expp", bufs=2))
        rdp = ctx.enter_context(tc.tile_pool(name="rdp", bufs=2))
        htp = ctx.enter_context(tc.tile_pool(name="htp", bufs=2))
        xtp = ctx.enter_context(tc.tile_pool(name="xtp", bufs=3))
        obp = ctx.enter_context(tc.tile_pool(name="obp", bufs=3))

        tiles = {}  # isl -> (h_ps, den_ps), allocated lazily at first acc

        def scores_exp(isl, t):
            sc = scp.tile([P, 2, 512], F32, tag="sc")
            for pl in range(2):
                j = 2 * t + pl
                for c2 in range(2):
                    nc.tensor.matmul(
                        sc[:, pl, :],
                        lhsT=k_sb[:, 2 * c2:2 * c2 + 2, j * P:(j + 1) * P],
                        rhs=q_sb[:, 2 * c2:2 * c2 + 2, isl * ISL:(isl + 1) * ISL],
                        start=(c2 == 0), stop=(c2 == 1),
                        perf_mode=DR,
                    )
            ex = expp.tile([P, 2, 512], F8, tag="ex")
            nc.scalar.activation(out=ex, in_=sc,
                                 func=mybir.ActivationFunctionType.Exp,
                                 scale=float(C) ** -0.5, bias=ebias)
            return ex

        def acc_h_den(isl, t, ex):
            if isl not in tiles:
                h_t = hps.tile([P, 4, 512], F32, tag="h")
                den_t = dps.tile([P, 512], F32, tag="den")
                tiles[isl] = (h_t, den_t)
            h_ps, den_ps = tiles[isl]
            nc.tensor.matmul(
                den_ps, lhsT=ones2, rhs=ex,
                start=(t == 0), stop=(t == T - 1), perf_mode=DR,
            )
            for cc in range(4):
                nc.tensor.matmul(
                    h_ps[:, cc, :],
                    lhsT=v_sb[:, t, :, cc * P:(cc + 1) * P],
                    rhs=ex,
                    start=(t == 0), stop=(t == T - 1),
                    perf_mode=DR,
                )

        def epilogue(isl):
            h_ps, den_ps = tiles.pop(isl)
            rden = rdp.tile([P, 512], F32, tag="rden")
            nc.vector.reciprocal(rden, den_ps)
            hTn = htp.tile([P, 4, 512], BF16, tag="hTn")
            for cc in range(4):
                nc.vector.tensor_mul(hTn[:, cc, :], h_ps[:, cc, :], rden)
            for a in range(4):
                po = pop.tile([P, 512], F32, tag="po")
                for cc in range(4):
                    nc.tensor.matmul(
                        po,
                        lhsT=hTn[:, cc, a * P:(a + 1) * P],
                        rhs=wo_sb[:, cc, :],
                        start=(cc == 0), stop=(cc == 3),
                    )
                ich = isl * 4 + a
                xt_sb = xtp.tile([P, C], F32, tag="xt_sb")
                nc.sync.dma_start(out=xt_sb, in_=xt_nc[ich * P:(ich + 1) * P, :])
                ob = obp.tile([P, C], F32, tag="ob")
                nc.vector.tensor_add(ob, po, xt_sb)
                nc.sync.dma_start(out=out_d[ich * P:(ich + 1) * P, :], in_=ob)

        # flat software pipeline over all (isl, t) pairs: emit scores(t+1)
        # before h/den(t) so the PE computes scores while ACT runs exp; the
        # per-isl epilogue is emitted right after its last h/den lands.
        pairs = [(isl, t) for isl in range(NISL) for t in range(T)]
        prev = None
        for cur in pairs:
            ex_cur = scores_exp(*cur)
            if prev is not None:
                acc_h_den(prev[0], prev[1], ex_prev)
                if prev[1] == T - 1:
                    epilogue(prev[0])
            prev, ex_prev = cur, ex_cur
        acc_h_den(prev[0], prev[1], ex_prev)
        epilogue(prev[0])

    _split_waits(nc)
    return nc


_GRAPH = None


def _f8(a):
    return np.asarray(a, dtype=np.float32).astype(ml_dtypes.float8_e4m3)


def kernel(**inputs):
    global _GRAPH, LAST_RESULT
    x = np.ascontiguousarray(np.asarray(inputs["x"], dtype=np.float32))
    v_qkv = np.asarray(inputs["v_qkv"], dtype=np.float32)
    g_qkv = np.asarray(inputs["g_qkv"], dtype=np.float32)
    v_out = np.asarray(inputs["v_out"], dtype=np.float32)
    g_out = np.asarray(inputs["g_out"], dtype=np.float32)

    # weight norm on host
    w_qkv = (g_qkv[:, None] * v_qkv
             / np.linalg.norm(v_qkv.astype(np.float64), axis=1, keepdims=True)
             ).astype(np.float32)  # [3C, C]
    w_out = (g_out[:, None] * v_out
             / np.linalg.norm(v_out.astype(np.float64), axis=1, keepdims=True)
             ).astype(np.float32)  # [C, C]

    # [128 p, 2 c2, 2 pl, O3]: w_pack[p,c2,pl,o] = S * w_qkv[o, c2*256+pl*128+p]
    wq = (S * w_qkv.T).reshape(2, 2, P, O3)
    w_pack = _f8(np.ascontiguousarray(wq.transpose(2, 0, 1, 3)))
    # Gram matrix for the RMS sum-of-squares: same c_in packing as w_pack
    G = (SG * (w_qkv.T @ w_qkv)).reshape(2, 2, P, C)
    g_pack = _f8(np.ascontiguousarray(G.transpose(2, 0, 1, 3)))
    # [128 p, 4 cc, C]: wo_pack[p,cc,o] = 2^-0.5 * w_out[o, cc*128+p]
    wo = (np.float32(2.0 ** -0.5) * w_out.T).reshape(4, P, C)
    wo_pack = np.ascontiguousarray(wo.transpose(1, 0, 2)).astype(ml_dtypes.bfloat16)
    ident_bf = np.eye(P, dtype=ml_dtypes.bfloat16)
    rsqrt2 = np.float32(2.0 ** -0.5)

    xt = x.reshape(B, C, N)
    in_maps = []
    for core in range(8):
        b, h = core // 2, core % 2
        if h == 0:
            x_perm = xt[b]
        else:
            x_perm = np.concatenate([xt[b][:, NH:], xt[b][:, :NH]], axis=1)
        x_perm = np.ascontiguousarray(x_perm)
        x_pack = np.ascontiguousarray(
            x_perm.reshape(4, P, N).transpose(1, 0, 2))  # [128, 4cc, N]
        xn_pack = np.ascontiguousarray(
            x_perm.T.reshape(NCH, P, C).transpose(1, 0, 2)
        ).astype(ml_dtypes.bfloat16)  # [128 p, nch, C]: x_perm[c, nch*128+p]
        in_maps.append({
            "x_pack": _f8(x_pack),
            "xn_pack": xn_pack,
            "w_pack": w_pack,
            "g_pack": g_pack,
            "wo_pack": wo_pack,
            "ident_bf": ident_bf,
            "xt_nc": np.ascontiguousarray(x_perm[:, :NH].T * rsqrt2),
        })

    if _GRAPH is None:
        _GRAPH = build_graph()

    res = run_bass_kernel_spmd(_GRAPH, in_maps, core_ids=list(range(8)))
    LAST_RESULT = res

    out = np.empty((B, C, N), np.float32)
    for core in range(8):
        b, h = core // 2, core % 2
        out[b][:, h * NH:(h + 1) * NH] = res.results[core]["out"].T
    return out.reshape(B, C, 64, 64)
